# revision 6
# baseline (speedup 1.0000x reference)
# Trainium2 Bass kernel for nn_CrossAttention (B=1, I=J=1024, C_S=1024,
# C_Z=128, H=16, D=64), sharded over the query dim i across 8 NeuronCores.
#
# Per-core program (i-slice of 128 query rows):
#   qT = (Wq s_c^T + bq)/sqrt(D)  kT = Wk k_in^T   v = k_in Wv^T  (bf16 matmuls)
#   z[i,j,h] = sum_c bias[i,j,c] Wz[c,h]   via PE-transpose of bias [i,c] blocks
#              (bias cast to bf16 during DMA) + per-j matmuls with Wz
#   qk[i,j] = qT_h^T kT_h  (PSUM), z added in-place in PSUM, exp on ACT
#              (softmax over j without max-subtraction; logits are O(1))
#   o[i,:] = sum_j exp^T v_aug[j]  with mask[j] in an extra v column so the
#              denominator comes out of the same matmul chain
#   out = (sigmoid(s_c Wg^T) * o) @ Wo^T
#
# The z-path (1024 PE transposes + 1024 small matmuls) is interleaved at fine
# granularity with the projection matmuls and the 4 attention passes (256 j
# each) so the tensor engine never idles and the HAM clock stays warm, while
# the 64 MB/core bias DMA streams continuously underneath.
#
# kernel(**inputs) takes FULL inputs, shards on host, runs SPMD on cores 0-7,
# gathers to the full [1, 1024, 1024] output.

import numpy as np

B, I, J, CS, CZ, H, D = 1, 1024, 1024, 1024, 128, 16, 64
NCORES = 8
NI = I // NCORES  # 128 query rows per core
P = 128
NCHUNK = 32  # bias chunks of 32 j
CJ = J // NCHUNK  # 32 j per chunk
NUNIT = J // 8  # z work units of 8 j

_last_results = None


def _build_program():
    from contextlib import ExitStack

    import concourse.mybir as mybir
    import concourse.tile as tile
    from concourse import bacc
    from concourse.masks import make_identity

    f32 = mybir.dt.float32
    bf16 = mybir.dt.bfloat16
    AF = mybir.ActivationFunctionType
    ALU = mybir.AluOpType

    nc = bacc.Bacc("TRN2", target_bir_lowering=False, debug=False)

    # ---- dram io ----
    s_c = nc.dram_tensor("s_c", [NI, CS], f32, kind="ExternalInput").ap()
    bias_c = nc.dram_tensor("bias_c", [NI, J, CZ], f32, kind="ExternalInput").ap()
    k_in = nc.dram_tensor("k_in", [J, CS], f32, kind="ExternalInput").ap()
    mask = nc.dram_tensor("mask", [J], f32, kind="ExternalInput").ap()
    w_q = nc.dram_tensor("w_q", [CS, CS], f32, kind="ExternalInput").ap()
    w_k = nc.dram_tensor("w_k", [CS, CS], f32, kind="ExternalInput").ap()
    w_v = nc.dram_tensor("w_v", [CS, CS], f32, kind="ExternalInput").ap()
    w_g = nc.dram_tensor("w_g", [CS, CS], f32, kind="ExternalInput").ap()
    w_o = nc.dram_tensor("w_o", [CS, CS], f32, kind="ExternalInput").ap()
    b_q = nc.dram_tensor("b_q", [CS], f32, kind="ExternalInput").ap()
    w_z = nc.dram_tensor("w_z", [CZ, H], f32, kind="ExternalInput").ap()
    out_d = nc.dram_tensor("out", [NI, CS], f32, kind="ExternalOutput").ap()

    KC = CS // P  # 8 contraction chunks

    with tile.TileContext(nc) as tc, ExitStack() as ctx:
        pool = lambda name, bufs: ctx.enter_context(tc.tile_pool(name=name, bufs=bufs))
        ppool = lambda name, bufs: ctx.enter_context(
            tc.tile_pool(name=name, bufs=bufs, space="PSUM")
        )

        const = pool("const", 1)
        wnat_p = pool("wnat", 2)
        wt_p = pool("wt", 2)
        kin_p = pool("kin", 1)
        small_p = pool("small", 1)
        big_p = pool("big", 1)
        bstage_p = pool("bstage", 2)
        bt_p = pool("bt", 3)
        et_p = pool("et", 2)
        r_p = pool("r", 2)
        outs_p = pool("outs", 1)

        tpsum = ppool("tpsum", 2)  # transpose targets (1 bank each)
        zps = ppool("zps", 1)  # z accumulation [128,512] f32
        bigps = ppool("bigps", 2)  # projection accumulators
        qkps = ppool("qkps", 2)  # attention logits [128,256] f32
        ops = ppool("ops", 1)  # attention output [128,<=512] f32

        engflip = [0]

        def copy_alt(out, in_):
            # alternate PSUM evacuations between DVE and ACT
            engflip[0] ^= 1
            if engflip[0]:
                nc.vector.tensor_copy(out, in_)
            else:
                nc.scalar.copy(out, in_)

        ident = const.tile([P, P], bf16)
        make_identity(nc, ident)
        wz_s = const.tile([CZ, H], bf16)
        nc.gpsimd.dma_start(wz_s, w_z)  # cast f32 -> bf16
        bq_s = const.tile([P, KC], f32)
        nc.sync.dma_start(bq_s, b_q.rearrange("(fo p) -> p fo", p=P))
        mask_s = const.tile([P, KC], f32)
        nc.sync.dma_start(mask_s, mask.rearrange("(jo p) -> p jo", p=P))

        # ---- z state ----
        # z_s layout: [i_part, chunk(16), j_local(64), h(16)] bf16
        z_s = big_p.tile([P, NCHUNK, CJ, H], bf16, tag="z")
        bstage = [None] * NCHUNK

        def emit_chunk_dma(c):
            bt = bstage_p.tile([P, CJ, CZ], bf16, tag="bt", name=f"bt_{c}")
            bstage[c] = bt
            if c == 0:
                # split the first chunk so the PE can start early
                nc.gpsimd.dma_start(bt[:, :16, :], bias_c[:, :16, :])
                nc.gpsimd.dma_start(bt[:, 16:, :], bias_c[:, 16:CJ, :])
            else:
                nc.gpsimd.dma_start(bt, bias_c[:, c * CJ : (c + 1) * CJ, :])

        zstate = {"u": 0, "zp": None}

        def emit_z_unit():
            # one unit = 8 j: 8 transposes -> bT copy -> 8 z-matmuls
            u = zstate["u"]
            assert u < NUNIT
            zstate["u"] = u + 1
            c, uu = divmod(u, 4)  # chunk, unit-in-chunk
            if uu == 0:
                if bstage[c] is None:
                    emit_chunk_dma(c)
                if c + 1 < NCHUNK and bstage[c + 1] is None:
                    emit_chunk_dma(c + 1)
                zstate["zp"] = zps.tile([P, 512], f32, tag="zp", name=f"zp_{u}")
            bt = bstage[c]
            tb = tpsum.tile([P, 1024], bf16, tag="tb", name=f"ztb_{u}")
            for jl in range(8):
                nc.tensor.transpose(
                    tb[:, jl * P : (jl + 1) * P], bt[:, uu * 8 + jl, :], ident
                )
            bT = bt_p.tile([P, 8, P], bf16, tag="bT", name=f"bT_{u}")
            copy_alt(bT, tb)
            zp = zstate["zp"]
            for jl in range(8):
                jj = uu * 8 + jl
                nc.tensor.matmul(
                    zp[:, jj * H : (jj + 1) * H],
                    bT[:, jl, :],
                    wz_s,
                    start=True,
                    stop=True,
                )
            if uu == 3:
                nc.vector.tensor_copy(
                    z_s[:, c, :, :].rearrange("p a b -> p (a b)"), zp
                )

        def zsteps(n):
            for _ in range(min(n, NUNIT - zstate["u"])):
                emit_z_unit()

        # ---- kick off input DMAs in priority order ----
        emit_chunk_dma(0)
        snat = small_p.tile([P, CS], bf16, tag="snat")
        nc.gpsimd.dma_start(snat, s_c)
        knat = kin_p.tile([P, KC, CS], bf16, tag="knat")
        kr = k_in.rearrange("(jo p) c -> p jo c", p=P)
        nc.gpsimd.dma_start(knat[:, :4, :], kr[:, :4, :])
        nc.gpsimd.dma_start(knat[:, 4:, :], kr[:, 4:, :])

        # ---- sT: transpose s_c to [c, i] ----
        sT = small_p.tile([P, KC, NI], bf16, tag="sT")
        for ch in range(2):
            tb = tpsum.tile([P, 1024], bf16, tag="tb", name=f"stb_{ch}")
            for co in range(ch * 4, ch * 4 + 4):
                nc.tensor.transpose(
                    tb[:, (co % 4) * P : (co % 4 + 1) * P],
                    snat[:, co * P : (co + 1) * P],
                    ident,
                )
            nc.vector.tensor_copy(sT[:, ch * 4 : (ch + 1) * 4, :], tb[:, : 4 * P])

        zsteps(2)

        # ---- weights: load natural [f,c] bf16, PE-transpose to [c,f] ----
        # one fh group = 2 fo rows of 128 f: 16 transposes -> 2 copies
        def load_w_nat(w_ap, name):
            wr = w_ap.rearrange("(fo p) c -> p fo c", p=P)
            wnat = wnat_p.tile([P, KC, CS], bf16, tag="wnat", name=name)
            nc.gpsimd.dma_start(wnat[:, :4, :], wr[:, :4, :])
            nc.gpsimd.dma_start(wnat[:, 4:, :], wr[:, 4:, :])
            return wnat

        def emit_wT_group(wnat, wT, fh):
            # transposes [f,c]->[c,f] for f rows fh*256..fh*256+255
            for fo in range(fh * 2, fh * 2 + 2):
                tb = tpsum.tile([P, 1024], bf16, tag="tb", name=f"wtb_{wT.name}_{fo}")
                for co in range(KC):
                    nc.tensor.transpose(
                        tb[:, co * P : (co + 1) * P],
                        wnat[:, fo, co * P : (co + 1) * P],
                        ident,
                    )
                # tb holds [c_sub, co(8) x 128f]; scatter to wT[:, co, fo*128..]
                copy_alt(
                    wT[:, :, fo * P : (fo + 1) * P],
                    tb.rearrange("p (a b) -> p a b", a=KC),
                )

        # ---- Wq + Q projection: qT[f,i] = (Wq s^T + bq)/sqrt(D) ----
        wq_nat = load_w_nat(w_q, "wq")
        wqT = wt_p.tile([P, KC, CS], bf16, tag="wt", name="wqT")
        qT = small_p.tile([P, KC, NI], bf16, tag="qT")
        for fh in range(4):
            emit_wT_group(wq_nat, wqT, fh)
            zsteps(1)
        for fo in range(KC):
            ps = bigps.tile([P, 512], f32, tag="big", name=f"qp_{fo}")
            for co in range(KC):
                nc.tensor.matmul(
                    ps[:, :NI],
                    wqT[:, co, fo * P : (fo + 1) * P],
                    sT[:, co, :],
                    start=(co == 0),
                    stop=(co == KC - 1),
                )
            nc.vector.tensor_scalar(
                qT[:, fo, :],
                ps[:, :NI],
                bq_s[:, fo : fo + 1],
                1.0 / np.sqrt(D),
                ALU.add,
                ALU.mult,
            )
            if fo % 2 == 1:
                zsteps(1)

        # ---- kinT: transpose k_in to [c, j] ----
        kinT = kin_p.tile([P, KC, J], bf16, tag="kinT")
        for co in range(KC):
            tb = tpsum.tile([P, 1024], bf16, tag="tb", name=f"ktb_{co}")
            for jo in range(KC):
                nc.tensor.transpose(
                    tb[:, jo * P : (jo + 1) * P],
                    knat[:, jo, co * P : (co + 1) * P],
                    ident,
                )
            copy_alt(kinT[:, co, :], tb)
            if co % 2 == 1:
                zsteps(1)

        # ---- Wk + K projection: kT[f,j] = Wk k_in^T ----
        wk_nat = load_w_nat(w_k, "wk")
        wkT = wt_p.tile([P, KC, CS], bf16, tag="wt", name="wkT")
        kT = big_p.tile([P, KC, J], bf16, tag="kT")
        for fh in range(4):
            emit_wT_group(wk_nat, wkT, fh)
            zsteps(1)

        def emit_k_proj(fo, jh):
            ps = bigps.tile([P, 512], f32, tag="big", name=f"kp_{fo}_{jh}")
            for co in range(KC):
                nc.tensor.matmul(
                    ps,
                    wkT[:, co, fo * P : (fo + 1) * P],
                    kinT[:, co, jh * 512 : (jh + 1) * 512],
                    start=(co == 0),
                    stop=(co == KC - 1),
                )
            copy_alt(kT[:, fo, jh * 512 : (jh + 1) * 512], ps)

        for fo in range(KC):  # j half 0 first: unblocks attn passes 0-1
            emit_k_proj(fo, 0)
            zsteps(1)

        # ---- Wv + V projection (j-half 0): v[j, h, d|mask] ----
        wv_nat = load_w_nat(w_v, "wv")
        wvT = wt_p.tile([P, KC, CS], bf16, tag="wt", name="wvT")
        v_s = big_p.tile([P, KC, H, D + 1], bf16, tag="v")
        for fh in range(4):
            emit_wT_group(wv_nat, wvT, fh)
            zsteps(1)

        def emit_v_proj(jo):
            for fh in range(2):
                ps = bigps.tile([P, 512], f32, tag="big", name=f"vp_{jo}_{fh}")
                for co in range(KC):
                    nc.tensor.matmul(
                        ps,
                        kinT[:, co, jo * P : (jo + 1) * P],
                        wvT[:, co, fh * 512 : (fh + 1) * 512],
                        start=(co == 0),
                        stop=(co == KC - 1),
                    )
                nc.vector.tensor_scalar_mul(
                    v_s[:, jo, fh * 8 : (fh + 1) * 8, 0:D],
                    ps,
                    mask_s[:, jo : jo + 1],
                )
            nc.vector.tensor_copy(
                v_s[:, jo, :, D : D + 1],
                mask_s[:, jo : jo + 1, None].to_broadcast((P, H, 1)),
            )

        for jo in range(4):
            emit_v_proj(jo)
            zsteps(1)

        # ---- attention pass machinery: 4 passes of 256 j ----
        o_s = small_p.tile([P, CS], bf16, tag="o")
        o_acc = small_p.tile([P, H, D + 1], f32, tag="oacc")

        def emit_attn_h(jq, h):
            fo, pb = h // 2, (h % 2) * D
            qkt = qkps.tile([P, 512], f32, tag="qk", name=f"qk_{jq}_{h}")
            qk = qkt[:, :256]
            nc.tensor.matmul(
                qk,
                qT[pb : pb + D, fo, :],
                kT[pb : pb + D, fo, jq * 256 : (jq + 1) * 256],
                start=True,
                stop=True,
            )
            # add z in place in PSUM, then exp on ACT reading PSUM
            nc.vector.tensor_tensor(
                qk,
                qk,
                z_s[:, 8 * jq : 8 * (jq + 1), :, h].rearrange("p a b -> p (a b)"),
                ALU.add,
            )
            et = et_p.tile([P, 256], bf16, tag="et", name=f"et_{jq}_{h}")
            nc.scalar.activation(et, qk, AF.Exp)
            tb = tpsum.tile([P, 1024], bf16, tag="tb", name=f"etb_{jq}_{h}")
            for jl in range(2):
                nc.tensor.transpose(
                    tb[:, jl * P : (jl + 1) * P], et[:, jl * P : (jl + 1) * P], ident
                )
            etT = et_p.tile([P, 2, P], bf16, tag="etT", name=f"etT_{jq}_{h}")
            copy_alt(etT, tb[:, : 2 * P])
            op = ops.tile([P, 512], f32, tag="op", name=f"op_{jq}_{h}")
            for q2 in range(2):
                nc.tensor.matmul(
                    op[:, : D + 1],
                    etT[:, q2, :],
                    v_s[:, jq * 2 + q2, h, :],
                    start=(q2 == 0),
                    stop=(q2 == 1),
                )
            if jq == 0:
                nc.vector.tensor_copy(o_acc[:, h, :], op[:, : D + 1])
            else:
                nc.vector.tensor_tensor(
                    o_acc[:, h, :], op[:, : D + 1], o_acc[:, h, :], ALU.add
                )

        def emit_attn_pass(jq):
            for h in range(H):
                emit_attn_h(jq, h)
                if h % 2 == 1:
                    zsteps(1)

        # pass 0 needs z chunks 0-3 (units 0-31), kT jh0, v jo0-1, qT
        while zstate["u"] < 32:
            emit_z_unit()
        emit_attn_pass(0)

        # ---- K projection j-half 1 ----
        for fo in range(KC):
            emit_k_proj(fo, 1)
            zsteps(1)

        # ---- V projection jo 4-7 ----
        for jo in range(4, 8):
            emit_v_proj(jo)
            zsteps(1)

        # ---- Wg + G projection: g = sigmoid(s Wg^T) ----
        wg_nat = load_w_nat(w_g, "wg")
        wgT = wt_p.tile([P, KC, CS], bf16, tag="wt", name="wgT")
        g_s = small_p.tile([P, CS], bf16, tag="g")
        for fh in range(4):
            emit_wT_group(wg_nat, wgT, fh)
            zsteps(1)
        for fh in range(2):
            ps = bigps.tile([P, 512], f32, tag="big", name=f"gp_{fh}")
            for co in range(KC):
                nc.tensor.matmul(
                    ps,
                    sT[:, co, :],
                    wgT[:, co, fh * 512 : (fh + 1) * 512],
                    start=(co == 0),
                    stop=(co == KC - 1),
                )
            nc.scalar.activation(g_s[:, fh * 512 : (fh + 1) * 512], ps, AF.Sigmoid)
            zsteps(1)

        # pass 1 needs z chunks 4-7 (units 32-63), kT jh1, v jo2-3
        while zstate["u"] < 64:
            emit_z_unit()
        emit_attn_pass(1)

        # ---- Wo transpose (needed only at the tail) ----
        wo_nat = load_w_nat(w_o, "wo")
        woT = wt_p.tile([P, KC, CS], bf16, tag="wt", name="woT")
        for fh in range(4):
            emit_wT_group(wo_nat, woT, fh)
            zsteps(2)

        # pass 2 needs z chunks 8-11 (units 64-95)
        while zstate["u"] < 96:
            emit_z_unit()
        emit_attn_pass(2)

        # remaining z units (96-127), then final pass
        while zstate["u"] < NUNIT:
            emit_z_unit()
        emit_attn_pass(3)

        # ---- normalize: o = o_num / o_den ----
        for h in range(H):
            rec = r_p.tile([P, 1], f32, tag="r", name=f"rec_{h}")
            nc.vector.reciprocal(rec, o_acc[:, h, D : D + 1])
            nc.vector.tensor_scalar_mul(
                o_s[:, h * D : (h + 1) * D], o_acc[:, h, 0:D], rec
            )

        # ---- gating + output projection ----
        nc.vector.tensor_mul(g_s, g_s, o_s)
        goT = small_p.tile([P, KC, NI], bf16, tag="goT")
        for gh in range(2):
            tb = tpsum.tile([P, 1024], bf16, tag="tb", name=f"gtb_{gh}")
            for fo in range(gh * 4, gh * 4 + 4):
                nc.tensor.transpose(
                    tb[:, (fo % 4) * P : (fo % 4 + 1) * P],
                    g_s[:, fo * P : (fo + 1) * P],
                    ident,
                )
            nc.vector.tensor_copy(goT[:, gh * 4 : (gh + 1) * 4, :], tb[:, : 4 * P])

        for fh in range(2):
            ps = bigps.tile([P, 512], f32, tag="big", name=f"op_ps_{fh}")
            for fo in range(KC):
                nc.tensor.matmul(
                    ps,
                    goT[:, fo, :],
                    woT[:, fo, fh * 512 : (fh + 1) * 512],
                    start=(fo == 0),
                    stop=(fo == KC - 1),
                )
            out_s = outs_p.tile([P, 512], f32, tag="outs", name=f"out_s{fh}")
            nc.vector.tensor_copy(out_s, ps)
            nc.sync.dma_start(out_d[:, fh * 512 : (fh + 1) * 512], out_s)

    nc.compile()
    return nc


def kernel(**inputs):
    global _last_results
    from concourse.bass_utils import run_bass_kernel_spmd

    s = np.ascontiguousarray(np.asarray(inputs["s"], dtype=np.float32)[0])
    k_in = np.ascontiguousarray(np.asarray(inputs["k_in"], dtype=np.float32)[0])
    mask = np.ascontiguousarray(np.asarray(inputs["mask"], dtype=np.float32)[0])
    bias = np.asarray(inputs["bias"], dtype=np.float32)[0]
    wq = np.ascontiguousarray(np.asarray(inputs["Wq"], dtype=np.float32))
    wk = np.ascontiguousarray(np.asarray(inputs["Wk"], dtype=np.float32))
    wv = np.ascontiguousarray(np.asarray(inputs["Wv"], dtype=np.float32))
    wg = np.ascontiguousarray(np.asarray(inputs["Wg"], dtype=np.float32))
    wo = np.ascontiguousarray(np.asarray(inputs["Wo"], dtype=np.float32))
    bq = np.ascontiguousarray(np.asarray(inputs["bq"], dtype=np.float32))
    wz = np.ascontiguousarray(np.asarray(inputs["Wz"], dtype=np.float32))
    mult = int(np.asarray(inputs.get("multiplicity", 1)))
    assert mult == 1, f"multiplicity={mult} not supported (B=1)"

    nc = _build_program()

    in_maps = []
    for c in range(NCORES):
        in_maps.append(
            {
                "s_c": np.ascontiguousarray(s[c * NI : (c + 1) * NI]),
                "bias_c": np.ascontiguousarray(bias[c * NI : (c + 1) * NI]),
                "k_in": k_in,
                "mask": mask,
                "w_q": wq,
                "w_k": wk,
                "w_v": wv,
                "w_g": wg,
                "w_o": wo,
                "b_q": bq,
                "w_z": wz,
            }
        )

    try:
        res = run_bass_kernel_spmd(nc, in_maps, core_ids=list(range(NCORES)))
    except Exception:
        # transient device-unrecoverable errors have been observed on a
        # first attempt; one retry has always succeeded
        import time as _time

        _time.sleep(5.0)
        res = run_bass_kernel_spmd(nc, in_maps, core_ids=list(range(NCORES)))
    _last_results = res
    out = np.concatenate([r["out"] for r in res.results], axis=0)
    return out.reshape(B, I, CS).astype(np.float32)


if __name__ == "__main__":
    rng = np.random.default_rng(0)
    ins = {
        "s": rng.standard_normal((B, I, CS), dtype=np.float32),
        "k_in": rng.standard_normal((B, J, CS), dtype=np.float32),
        "mask": np.ones((B, J), np.float32),
        "bias": rng.standard_normal((B, I, J, CZ), dtype=np.float32),
        "Wq": rng.standard_normal((CS, CS), dtype=np.float32) * 0.02,
        "bq": rng.standard_normal((CS,), dtype=np.float32) * 0.02,
        "Wk": rng.standard_normal((CS, CS), dtype=np.float32) * 0.02,
        "Wv": rng.standard_normal((CS, CS), dtype=np.float32) * 0.02,
        "Wg": rng.standard_normal((CS, CS), dtype=np.float32) * 0.02,
        "Wo": rng.standard_normal((CS, CS), dtype=np.float32) * 0.02,
        "Wz": rng.standard_normal((CZ, H), dtype=np.float32) * 0.02,
        "multiplicity": 1,
    }
    out = kernel(**ins)
    print(out.shape, out.dtype)


# revision 7
# speedup vs baseline: 1.1864x; 1.1864x over previous
# Trainium2 Bass kernel for nn_CrossAttention (B=1, I=J=1024, C_S=1024,
# C_Z=128, H=16, D=64), sharded over the query dim i across 8 NeuronCores.
#
# Per-core program (i-slice of 128 query rows):
#   qT = (Wq s_c^T + bq)/sqrt(D)  kT = Wk k_in^T   v = k_in Wv^T  (bf16 matmuls)
#   z[i,j,h] = sum_c bias[i,j,c] Wz[c,h]   via PE-transpose of bias [i,c] blocks
#              (bias cast to bf16 during DMA) + per-j matmuls with Wz
#   qk[i,j] = qT_h^T kT_h  (PSUM), z added in-place in PSUM, exp on ACT
#              (softmax over j without max-subtraction; logits are O(1))
#   o[i,:] = sum_j exp^T v_aug[j]  with mask[j] in an extra v column so the
#              denominator comes out of the same matmul chain
#   out = (sigmoid(s_c Wg^T) * o) @ Wo^T
#
# The z-path (1024 PE transposes + 1024 small matmuls) is interleaved at fine
# granularity with the projection matmuls and 3 attention passes (512/256/256
# j), while the 64 MB/core bias DMA streams continuously underneath.  Weight
# DMAs are issued just-in-time in 2 MB halves so they do not starve the bias
# stream early on.
#
# kernel(**inputs) takes FULL inputs, shards on host, runs SPMD on cores 0-7,
# gathers to the full [1, 1024, 1024] output.

import numpy as np

B, I, J, CS, CZ, H, D = 1, 1024, 1024, 1024, 128, 16, 64
NCORES = 8
NI = I // NCORES  # 128 query rows per core
P = 128
NCHUNK = 32  # bias chunks of 32 j
CJ = J // NCHUNK  # 32 j per chunk
NUNIT = J // 8  # z work units of 8 j

_last_results = None


def _build_program():
    from contextlib import ExitStack

    import concourse.mybir as mybir
    import concourse.tile as tile
    from concourse import bacc
    from concourse.masks import make_identity

    f32 = mybir.dt.float32
    bf16 = mybir.dt.bfloat16
    AF = mybir.ActivationFunctionType
    ALU = mybir.AluOpType

    nc = bacc.Bacc("TRN2", target_bir_lowering=False, debug=False)

    # ---- dram io ----
    s_c = nc.dram_tensor("s_c", [NI, CS], f32, kind="ExternalInput").ap()
    bias_c = nc.dram_tensor("bias_c", [NI, J, CZ], f32, kind="ExternalInput").ap()
    k_in = nc.dram_tensor("k_in", [J, CS], f32, kind="ExternalInput").ap()
    mask = nc.dram_tensor("mask", [J], f32, kind="ExternalInput").ap()
    w_q = nc.dram_tensor("w_q", [CS, CS], f32, kind="ExternalInput").ap()
    w_k = nc.dram_tensor("w_k", [CS, CS], f32, kind="ExternalInput").ap()
    w_v = nc.dram_tensor("w_v", [CS, CS], f32, kind="ExternalInput").ap()
    w_g = nc.dram_tensor("w_g", [CS, CS], f32, kind="ExternalInput").ap()
    w_o = nc.dram_tensor("w_o", [CS, CS], f32, kind="ExternalInput").ap()
    b_q = nc.dram_tensor("b_q", [CS], f32, kind="ExternalInput").ap()
    w_z = nc.dram_tensor("w_z", [CZ, H], f32, kind="ExternalInput").ap()
    out_d = nc.dram_tensor("out", [NI, CS], f32, kind="ExternalOutput").ap()

    KC = CS // P  # 8 contraction chunks

    with tile.TileContext(nc) as tc, ExitStack() as ctx:
        pool = lambda name, bufs: ctx.enter_context(tc.tile_pool(name=name, bufs=bufs))
        ppool = lambda name, bufs: ctx.enter_context(
            tc.tile_pool(name=name, bufs=bufs, space="PSUM")
        )

        const = pool("const", 1)
        wnat_p = pool("wnat", 2)
        wt_p = pool("wt", 2)
        kin_p = pool("kin", 1)
        small_p = pool("small", 1)
        big_p = pool("big", 1)
        bstage_p = pool("bstage", 3)
        bt_p = pool("bt", 3)
        et_p = pool("et", 2)
        r_p = pool("r", 2)
        outs_p = pool("outs", 1)

        tpsum = ppool("tpsum", 2)  # transpose targets (1 bank each)
        zps = ppool("zps", 1)  # z accumulation [128,512] f32
        bigps = ppool("bigps", 2)  # projection accumulators
        qkps = ppool("qkps", 2)  # attention logits f32
        ops = ppool("ops", 1)  # attention output f32

        # alternate big PSUM evacuations 2:1 between DVE and ACT (ACT SBUF
        # writes are ~1.9x slower than DVE's)
        engflip = [0]

        def copy_alt(out, in_):
            engflip[0] = (engflip[0] + 1) % 3
            if engflip[0] == 0:
                nc.scalar.copy(out, in_)
            else:
                nc.vector.tensor_copy(out, in_)

        ident = const.tile([P, P], bf16)
        make_identity(nc, ident)
        wz_s = const.tile([CZ, H], bf16)
        nc.gpsimd.dma_start(wz_s, w_z)  # cast f32 -> bf16
        bq_s = const.tile([P, KC], f32)
        nc.sync.dma_start(bq_s, b_q.rearrange("(fo p) -> p fo", p=P))
        mask_s = const.tile([P, KC], f32)
        nc.sync.dma_start(mask_s, mask.rearrange("(jo p) -> p jo", p=P))

        # ---- z state ----
        # z_s layout: [i_part, chunk(32), j_local(32), h(16)] bf16
        z_s = big_p.tile([P, NCHUNK, CJ, H], bf16, tag="z")
        bstage = [None] * NCHUNK

        def emit_chunk_dma(c):
            bt = bstage_p.tile([P, CJ, CZ], bf16, tag="bt", name=f"bt_{c}")
            bstage[c] = bt
            if c == 0:
                # split the first chunk so the PE can start early
                nc.gpsimd.dma_start(bt[:, :16, :], bias_c[:, :16, :])
                nc.gpsimd.dma_start(bt[:, 16:, :], bias_c[:, 16:CJ, :])
            else:
                nc.gpsimd.dma_start(bt, bias_c[:, c * CJ : (c + 1) * CJ, :])

        zstate = {"u": 0, "zp": None}

        def emit_z_unit():
            # one unit = 8 j: 8 transposes -> bT copy -> 8 z-matmuls
            u = zstate["u"]
            assert u < NUNIT
            zstate["u"] = u + 1
            c, uu = divmod(u, 4)  # chunk, unit-in-chunk
            if uu == 0:
                if bstage[c] is None:
                    emit_chunk_dma(c)
                if c + 1 < NCHUNK and bstage[c + 1] is None:
                    emit_chunk_dma(c + 1)
                zstate["zp"] = zps.tile([P, 512], f32, tag="zp", name=f"zp_{u}")
            bt = bstage[c]
            tb = tpsum.tile([P, 1024], bf16, tag="tb", name=f"ztb_{u}")
            for jl in range(8):
                nc.tensor.transpose(
                    tb[:, jl * P : (jl + 1) * P], bt[:, uu * 8 + jl, :], ident
                )
            bT = bt_p.tile([P, 8, P], bf16, tag="bT", name=f"bT_{u}")
            copy_alt(bT, tb)
            zp = zstate["zp"]
            for jl in range(8):
                jj = uu * 8 + jl
                nc.tensor.matmul(
                    zp[:, jj * H : (jj + 1) * H],
                    bT[:, jl, :],
                    wz_s,
                    start=True,
                    stop=True,
                )
            if uu == 3:
                nc.vector.tensor_copy(
                    z_s[:, c, :, :].rearrange("p a b -> p (a b)"), zp
                )

        def zsteps(n):
            for _ in range(min(n, NUNIT - zstate["u"])):
                emit_z_unit()

        # ---- kick off: bias chunks first, then the small s DMA ----
        emit_chunk_dma(0)
        emit_chunk_dma(1)
        snat = small_p.tile([P, CS], bf16, tag="snat")
        nc.gpsimd.dma_start(snat, s_c)

        # ---- sT: transpose s_c to [c, i] ----
        sT = small_p.tile([P, KC, NI], bf16, tag="sT")
        for ch in range(2):
            tb = tpsum.tile([P, 1024], bf16, tag="tb", name=f"stb_{ch}")
            for co in range(ch * 4, ch * 4 + 4):
                nc.tensor.transpose(
                    tb[:, (co % 4) * P : (co % 4 + 1) * P],
                    snat[:, co * P : (co + 1) * P],
                    ident,
                )
            nc.vector.tensor_copy(sT[:, ch * 4 : (ch + 1) * 4, :], tb[:, : 4 * P])

        zsteps(4)

        # ---- weights: JIT 2 MB half-loads, PE-transpose [f,c]->[c,f] ----
        def emit_weight(w_ap, wT, name, zk=1):
            wr = w_ap.rearrange("(fo p) c -> p fo c", p=P)
            for half in range(2):
                wnat = wnat_p.tile([P, 4, CS], bf16, tag="wnat", name=f"{name}_{half}")
                nc.gpsimd.dma_start(wnat, wr[:, half * 4 : (half + 1) * 4, :])
                for fl in range(4):
                    fo = half * 4 + fl
                    tb = tpsum.tile([P, 1024], bf16, tag="tb", name=f"wtb_{name}_{fo}")
                    for co in range(KC):
                        nc.tensor.transpose(
                            tb[:, co * P : (co + 1) * P],
                            wnat[:, fl, co * P : (co + 1) * P],
                            ident,
                        )
                    # tb holds [c_sub, co(8) x 128f]; scatter to wT[:, co, fo*128..]
                    nc.vector.tensor_copy(
                        wT[:, :, fo * P : (fo + 1) * P],
                        tb.rearrange("p (a b) -> p a b", a=KC),
                    )
                    if fo % 2 == 1:
                        zsteps(zk)

        # ---- Wq + Q projection: qT[f,i] = (Wq s^T + bq)/sqrt(D) ----
        wqT = wt_p.tile([P, KC, CS], bf16, tag="wt", name="wqT")
        qT = small_p.tile([P, KC, NI], bf16, tag="qT")
        emit_weight(w_q, wqT, "wq")
        for fo in range(KC):
            ps = bigps.tile([P, 512], f32, tag="big", name=f"qp_{fo}")
            for co in range(KC):
                nc.tensor.matmul(
                    ps[:, :NI],
                    wqT[:, co, fo * P : (fo + 1) * P],
                    sT[:, co, :],
                    start=(co == 0),
                    stop=(co == KC - 1),
                )
            nc.vector.tensor_scalar(
                qT[:, fo, :],
                ps[:, :NI],
                bq_s[:, fo : fo + 1],
                1.0 / np.sqrt(D),
                ALU.add,
                ALU.mult,
            )
            if fo % 2 == 1:
                zsteps(1)

        # ---- kinT: transpose k_in to [c, j] ----
        knat = kin_p.tile([P, KC, CS], bf16, tag="knat")
        kr = k_in.rearrange("(jo p) c -> p jo c", p=P)
        nc.gpsimd.dma_start(knat[:, :4, :], kr[:, :4, :])
        nc.gpsimd.dma_start(knat[:, 4:, :], kr[:, 4:, :])
        kinT = kin_p.tile([P, KC, J], bf16, tag="kinT")
        for co in range(KC):
            tb = tpsum.tile([P, 1024], bf16, tag="tb", name=f"ktb_{co}")
            for jo in range(KC):
                nc.tensor.transpose(
                    tb[:, jo * P : (jo + 1) * P],
                    knat[:, jo, co * P : (co + 1) * P],
                    ident,
                )
            copy_alt(kinT[:, co, :], tb)
            if co % 2 == 1:
                zsteps(1)

        # ---- Wk + K projection: kT[f,j] = Wk k_in^T ----
        wkT = wt_p.tile([P, KC, CS], bf16, tag="wt", name="wkT")
        kT = big_p.tile([P, KC, J], bf16, tag="kT")
        emit_weight(w_k, wkT, "wk")

        def emit_k_proj(fo, jh):
            ps = bigps.tile([P, 512], f32, tag="big", name=f"kp_{fo}_{jh}")
            for co in range(KC):
                nc.tensor.matmul(
                    ps,
                    wkT[:, co, fo * P : (fo + 1) * P],
                    kinT[:, co, jh * 512 : (jh + 1) * 512],
                    start=(co == 0),
                    stop=(co == KC - 1),
                )
            copy_alt(kT[:, fo, jh * 512 : (jh + 1) * 512], ps)

        for fo in range(KC):  # j half 0 first: unblocks attn pass 0
            emit_k_proj(fo, 0)
            zsteps(1)

        # ---- Wv + V projection: v[j, h, d|mask] ----
        wvT = wt_p.tile([P, KC, CS], bf16, tag="wt", name="wvT")
        v_s = big_p.tile([P, KC, H, D + 1], bf16, tag="v")
        emit_weight(w_v, wvT, "wv")

        def emit_v_proj(jo):
            for fh in range(2):
                ps = bigps.tile([P, 512], f32, tag="big", name=f"vp_{jo}_{fh}")
                for co in range(KC):
                    nc.tensor.matmul(
                        ps,
                        kinT[:, co, jo * P : (jo + 1) * P],
                        wvT[:, co, fh * 512 : (fh + 1) * 512],
                        start=(co == 0),
                        stop=(co == KC - 1),
                    )
                nc.vector.tensor_scalar_mul(
                    v_s[:, jo, fh * 8 : (fh + 1) * 8, 0:D],
                    ps,
                    mask_s[:, jo : jo + 1],
                )
            nc.vector.tensor_copy(
                v_s[:, jo, :, D : D + 1],
                mask_s[:, jo : jo + 1, None].to_broadcast((P, H, 1)),
            )

        for jo in range(4):
            emit_v_proj(jo)
            zsteps(1)

        for fo in range(KC):
            emit_k_proj(fo, 1)
            zsteps(1)

        for jo in range(4, 8):
            emit_v_proj(jo)
            zsteps(1)

        # ---- Wg + G projection: g = sigmoid(s Wg^T) ----
        wgT = wt_p.tile([P, KC, CS], bf16, tag="wt", name="wgT")
        g_s = small_p.tile([P, CS], bf16, tag="g")
        emit_weight(w_g, wgT, "wg")
        for fh in range(2):
            ps = bigps.tile([P, 512], f32, tag="big", name=f"gp_{fh}")
            for co in range(KC):
                nc.tensor.matmul(
                    ps,
                    sT[:, co, :],
                    wgT[:, co, fh * 512 : (fh + 1) * 512],
                    start=(co == 0),
                    stop=(co == KC - 1),
                )
            nc.scalar.activation(g_s[:, fh * 512 : (fh + 1) * 512], ps, AF.Sigmoid)
            zsteps(1)

        # ---- Wo transpose (consumed only at the tail) ----
        woT = wt_p.tile([P, KC, CS], bf16, tag="wt", name="woT")
        emit_weight(w_o, woT, "wo", zk=2)

        # ---- attention: 3 passes over j (512, 256, 256) ----
        o_s = small_p.tile([P, CS], bf16, tag="o")
        o_acc = small_p.tile([P, H, D + 1], f32, tag="oacc")

        def emit_attn_h(jq, j0, nj, h, zk):
            fo, pb = h // 2, (h % 2) * D
            qkt = qkps.tile([P, 512], f32, tag="qk", name=f"qk_{jq}_{h}")
            qk = qkt[:, :nj]
            nc.tensor.matmul(
                qk,
                qT[pb : pb + D, fo, :],
                kT[pb : pb + D, fo, j0 : j0 + nj],
                start=True,
                stop=True,
            )
            # add z in place in PSUM, then exp on ACT reading PSUM
            nc.vector.tensor_tensor(
                qk,
                qk,
                z_s[:, j0 // CJ : (j0 + nj) // CJ, :, h].rearrange("p a b -> p (a b)"),
                ALU.add,
            )
            et = et_p.tile([P, 512], bf16, tag="et", name=f"et_{jq}_{h}")
            nc.scalar.activation(et[:, :nj], qk, AF.Exp)
            tb = tpsum.tile([P, 1024], bf16, tag="tb", name=f"etb_{jq}_{h}")
            nt = nj // P
            for jl in range(nt):
                nc.tensor.transpose(
                    tb[:, jl * P : (jl + 1) * P], et[:, jl * P : (jl + 1) * P], ident
                )
            etT = et_p.tile([P, 4, P], bf16, tag="etT", name=f"etT_{jq}_{h}")
            copy_alt(etT[:, :nt, :], tb[:, : nt * P])
            op = ops.tile([P, 512], f32, tag="op", name=f"op_{jq}_{h}")
            for q in range(nt):
                nc.tensor.matmul(
                    op[:, : D + 1],
                    etT[:, q, :],
                    v_s[:, j0 // P + q, h, :],
                    start=(q == 0),
                    stop=(q == nt - 1),
                )
            if jq == 0:
                nc.vector.tensor_copy(o_acc[:, h, :], op[:, : D + 1])
            else:
                nc.vector.tensor_tensor(
                    o_acc[:, h, :], op[:, : D + 1], o_acc[:, h, :], ALU.add
                )
            zsteps(zk)

        # pass 0: j 0..511 (needs z chunks 0-15, kT jh0, v jo0-3)
        while zstate["u"] < 64:
            emit_z_unit()
        for h in range(H):
            emit_attn_h(0, 0, 512, h, 2)
        # pass 1: j 512..767 (needs z chunks 16-23, kT jh1, v jo4-5)
        while zstate["u"] < 96:
            emit_z_unit()
        for h in range(H):
            emit_attn_h(1, 512, 256, h, 2)
        # pass 2: j 768..1023
        while zstate["u"] < NUNIT:
            emit_z_unit()
        for h in range(H):
            emit_attn_h(2, 768, 256, h, 0)

        # ---- normalize: o = o_num / o_den ----
        for h in range(H):
            rec = r_p.tile([P, 1], f32, tag="r", name=f"rec_{h}")
            nc.vector.reciprocal(rec, o_acc[:, h, D : D + 1])
            nc.vector.tensor_scalar_mul(
                o_s[:, h * D : (h + 1) * D], o_acc[:, h, 0:D], rec
            )

        # ---- gating + output projection ----
        nc.vector.tensor_mul(g_s, g_s, o_s)
        goT = small_p.tile([P, KC, NI], bf16, tag="goT")
        for gh in range(2):
            tb = tpsum.tile([P, 1024], bf16, tag="tb", name=f"gtb_{gh}")
            for fo in range(gh * 4, gh * 4 + 4):
                nc.tensor.transpose(
                    tb[:, (fo % 4) * P : (fo % 4 + 1) * P],
                    g_s[:, fo * P : (fo + 1) * P],
                    ident,
                )
            nc.vector.tensor_copy(goT[:, gh * 4 : (gh + 1) * 4, :], tb[:, : 4 * P])

        for fh in range(2):
            ps = bigps.tile([P, 512], f32, tag="big", name=f"op_ps_{fh}")
            for fo in range(KC):
                nc.tensor.matmul(
                    ps,
                    goT[:, fo, :],
                    woT[:, fo, fh * 512 : (fh + 1) * 512],
                    start=(fo == 0),
                    stop=(fo == KC - 1),
                )
            out_s = outs_p.tile([P, 512], f32, tag="outs", name=f"out_s{fh}")
            nc.vector.tensor_copy(out_s, ps)
            nc.sync.dma_start(out_d[:, fh * 512 : (fh + 1) * 512], out_s)

    nc.compile()
    return nc


def kernel(**inputs):
    global _last_results
    from concourse.bass_utils import run_bass_kernel_spmd

    s = np.ascontiguousarray(np.asarray(inputs["s"], dtype=np.float32)[0])
    k_in = np.ascontiguousarray(np.asarray(inputs["k_in"], dtype=np.float32)[0])
    mask = np.ascontiguousarray(np.asarray(inputs["mask"], dtype=np.float32)[0])
    bias = np.asarray(inputs["bias"], dtype=np.float32)[0]
    wq = np.ascontiguousarray(np.asarray(inputs["Wq"], dtype=np.float32))
    wk = np.ascontiguousarray(np.asarray(inputs["Wk"], dtype=np.float32))
    wv = np.ascontiguousarray(np.asarray(inputs["Wv"], dtype=np.float32))
    wg = np.ascontiguousarray(np.asarray(inputs["Wg"], dtype=np.float32))
    wo = np.ascontiguousarray(np.asarray(inputs["Wo"], dtype=np.float32))
    bq = np.ascontiguousarray(np.asarray(inputs["bq"], dtype=np.float32))
    wz = np.ascontiguousarray(np.asarray(inputs["Wz"], dtype=np.float32))
    mult = int(np.asarray(inputs.get("multiplicity", 1)))
    assert mult == 1, f"multiplicity={mult} not supported (B=1)"

    nc = _build_program()

    in_maps = []
    for c in range(NCORES):
        in_maps.append(
            {
                "s_c": np.ascontiguousarray(s[c * NI : (c + 1) * NI]),
                "bias_c": np.ascontiguousarray(bias[c * NI : (c + 1) * NI]),
                "k_in": k_in,
                "mask": mask,
                "w_q": wq,
                "w_k": wk,
                "w_v": wv,
                "w_g": wg,
                "w_o": wo,
                "b_q": bq,
                "w_z": wz,
            }
        )

    try:
        res = run_bass_kernel_spmd(nc, in_maps, core_ids=list(range(NCORES)))
    except Exception:
        # transient device-unrecoverable errors have been observed on a
        # first attempt; one retry has always succeeded
        import time as _time

        _time.sleep(5.0)
        res = run_bass_kernel_spmd(nc, in_maps, core_ids=list(range(NCORES)))
    _last_results = res
    out = np.concatenate([r["out"] for r in res.results], axis=0)
    return out.reshape(B, I, CS).astype(np.float32)


if __name__ == "__main__":
    rng = np.random.default_rng(0)
    ins = {
        "s": rng.standard_normal((B, I, CS), dtype=np.float32),
        "k_in": rng.standard_normal((B, J, CS), dtype=np.float32),
        "mask": np.ones((B, J), np.float32),
        "bias": rng.standard_normal((B, I, J, CZ), dtype=np.float32),
        "Wq": rng.standard_normal((CS, CS), dtype=np.float32) * 0.02,
        "bq": rng.standard_normal((CS,), dtype=np.float32) * 0.02,
        "Wk": rng.standard_normal((CS, CS), dtype=np.float32) * 0.02,
        "Wv": rng.standard_normal((CS, CS), dtype=np.float32) * 0.02,
        "Wg": rng.standard_normal((CS, CS), dtype=np.float32) * 0.02,
        "Wo": rng.standard_normal((CS, CS), dtype=np.float32) * 0.02,
        "Wz": rng.standard_normal((CZ, H), dtype=np.float32) * 0.02,
        "multiplicity": 1,
    }
    out = kernel(**ins)
    print(out.shape, out.dtype)


# revision 10
# speedup vs baseline: 1.2243x; 1.0319x over previous
# Trainium2 Bass kernel for nn_CrossAttention (B=1, I=J=1024, C_S=1024,
# C_Z=128, H=16, D=64), sharded over the query dim i across 8 NeuronCores.
#
# Per-core program (i-slice of 128 query rows):
#   qT = (Wq s_c^T + bq)/sqrt(D)  kT = Wk k_in^T   v = k_in Wv^T  (bf16 matmuls)
#   z[i,j,h] = sum_c bias[i,j,c] Wz[c,h]   via PE-transpose of bias [i,c] blocks
#              (bias cast to bf16 during DMA) + per-j matmuls with Wz
#   qk[i,j] = qT_h^T kT_h  (PSUM), z added in-place in PSUM, exp on ACT
#              (softmax over j without max-subtraction; logits are O(1))
#   o[i,:] = sum_j exp^T v_aug[j]  with mask[j] in an extra v column so the
#              denominator comes out of the same matmul chain
#   out = (sigmoid(s_c Wg^T) * o) @ Wo^T
#
# The z-path (1024 PE transposes + 1024 small matmuls) is interleaved at fine
# granularity with the projection matmuls and 3 attention passes (512/256/256
# j), while the 64 MB/core bias DMA streams continuously underneath.  Weight
# DMAs are issued just-in-time in 2 MB halves so they do not starve the bias
# stream early on.
#
# kernel(**inputs) takes FULL inputs, shards on host, runs SPMD on cores 0-7,
# gathers to the full [1, 1024, 1024] output.

import numpy as np

B, I, J, CS, CZ, H, D = 1, 1024, 1024, 1024, 128, 16, 64
NCORES = 8
NI = I // NCORES  # 128 query rows per core
P = 128
NCHUNK = 32  # bias chunks of 32 j
CJ = J // NCHUNK  # 32 j per chunk
NUNIT = J // 8  # z work units of 8 j

_last_results = None


def _build_program():
    from contextlib import ExitStack

    import concourse.mybir as mybir
    import concourse.tile as tile
    from concourse import bacc
    from concourse.masks import make_identity

    f32 = mybir.dt.float32
    bf16 = mybir.dt.bfloat16
    AF = mybir.ActivationFunctionType
    ALU = mybir.AluOpType

    nc = bacc.Bacc("TRN2", target_bir_lowering=False, debug=False)

    # ---- dram io ----
    s_c = nc.dram_tensor("s_c", [NI, CS], f32, kind="ExternalInput").ap()
    bias_c = nc.dram_tensor("bias_c", [NI, J, CZ], f32, kind="ExternalInput").ap()
    k_in = nc.dram_tensor("k_in", [J, CS], f32, kind="ExternalInput").ap()
    mask = nc.dram_tensor("mask", [J], f32, kind="ExternalInput").ap()
    w_q = nc.dram_tensor("w_q", [CS, CS], f32, kind="ExternalInput").ap()
    w_k = nc.dram_tensor("w_k", [CS, CS], f32, kind="ExternalInput").ap()
    w_v = nc.dram_tensor("w_v", [CS, CS], f32, kind="ExternalInput").ap()
    w_g = nc.dram_tensor("w_g", [CS, CS], f32, kind="ExternalInput").ap()
    w_o = nc.dram_tensor("w_o", [CS, CS], f32, kind="ExternalInput").ap()
    b_q = nc.dram_tensor("b_q", [CS], f32, kind="ExternalInput").ap()
    w_z = nc.dram_tensor("w_z", [CZ, H], f32, kind="ExternalInput").ap()
    out_d = nc.dram_tensor("out", [NI, CS], f32, kind="ExternalOutput").ap()

    KC = CS // P  # 8 contraction chunks

    with tile.TileContext(nc) as tc, ExitStack() as ctx:
        pool = lambda name, bufs: ctx.enter_context(tc.tile_pool(name=name, bufs=bufs))
        ppool = lambda name, bufs: ctx.enter_context(
            tc.tile_pool(name=name, bufs=bufs, space="PSUM")
        )

        const = pool("const", 1)
        wnat_p = pool("wnat", 2)
        wt_p = pool("wt", 2)
        kin_p = pool("kin", 1)
        small_p = pool("small", 1)
        big_p = pool("big", 1)
        bstage_p = pool("bstage", 3)
        bt_p = pool("bt", 3)
        et_p = pool("et", 2)
        r_p = pool("r", 2)
        outs_p = pool("outs", 1)

        tpsum = ppool("tpsum", 2)  # transpose targets (1 bank each)
        zps = ppool("zps", 1)  # z accumulation [128,512] f32
        bigps = ppool("bigps", 2)  # projection accumulators
        qkps = ppool("qkps", 2)  # attention logits f32
        ops = ppool("ops", 1)  # attention output f32

        # alternate big PSUM evacuations 2:1 between DVE and ACT (ACT SBUF
        # writes are ~1.9x slower than DVE's)
        engflip = [0]

        def copy_alt(out, in_):
            engflip[0] = (engflip[0] + 1) % 3
            if engflip[0] == 0:
                nc.scalar.copy(out, in_)
            else:
                nc.vector.tensor_copy(out, in_)

        ident = const.tile([P, P], bf16)
        make_identity(nc, ident)
        wz_s = const.tile([CZ, H], bf16)
        nc.gpsimd.dma_start(wz_s, w_z)  # cast f32 -> bf16
        bq_s = const.tile([P, KC], f32)
        nc.sync.dma_start(bq_s, b_q.rearrange("(fo p) -> p fo", p=P))
        mask_s = const.tile([P, KC], f32)
        nc.sync.dma_start(mask_s, mask.rearrange("(jo p) -> p jo", p=P))

        # ---- z state ----
        # z_s layout: [i_part, chunk(32), j_local(32), h(16)] bf16
        z_s = big_p.tile([P, NCHUNK, CJ, H], bf16, tag="z")
        bstage = [None] * NCHUNK

        def emit_chunk_dma(c):
            bt = bstage_p.tile([P, CJ, CZ], bf16, tag="bt", name=f"bt_{c}")
            bstage[c] = bt
            if c == 0:
                # split the first chunk so the PE can start early
                nc.gpsimd.dma_start(bt[:, :16, :], bias_c[:, :16, :])
                nc.gpsimd.dma_start(bt[:, 16:, :], bias_c[:, 16:CJ, :])
            else:
                nc.gpsimd.dma_start(bt, bias_c[:, c * CJ : (c + 1) * CJ, :])

        # software-pipelined z stream: transposes of unit u run while the
        # z-matmuls of unit u-1 (whose bT copy is by then complete) issue,
        # so the in-order PE queue never waits on a PSUM->SBUF copy.
        zstate = {"t_u": 0, "mm_u": 0, "zp": None, "bT": {}}

        def emit_z_transposes(u):
            c, uu = divmod(u, 4)  # chunk, unit-in-chunk
            if uu == 0:
                if bstage[c] is None:
                    emit_chunk_dma(c)
                if c + 1 < NCHUNK and bstage[c + 1] is None:
                    emit_chunk_dma(c + 1)
            bt = bstage[c]
            tb = tpsum.tile([P, 1024], bf16, tag="tb", name=f"ztb_{u}")
            for jl in range(8):
                nc.tensor.transpose(
                    tb[:, jl * P : (jl + 1) * P], bt[:, uu * 8 + jl, :], ident
                )
            bT = bt_p.tile([P, 8, P], bf16, tag="bT", name=f"bT_{u}")
            copy_alt(bT, tb)
            zstate["bT"][u] = bT

        def emit_z_mms(u):
            c, uu = divmod(u, 4)
            if uu == 0:
                zstate["zp"] = zps.tile([P, 512], f32, tag="zp", name=f"zp_{u}")
            zp = zstate["zp"]
            bT = zstate["bT"].pop(u)
            for jl in range(8):
                jj = uu * 8 + jl
                nc.tensor.matmul(
                    zp[:, jj * H : (jj + 1) * H],
                    bT[:, jl, :],
                    wz_s,
                    start=True,
                    stop=True,
                )
            if uu == 3:
                nc.scalar.copy(z_s[:, c, :, :].rearrange("p a b -> p (a b)"), zp)

        def zstep():
            if zstate["t_u"] < NUNIT:
                emit_z_transposes(zstate["t_u"])
                zstate["t_u"] += 1
            lag = 0 if zstate["t_u"] == NUNIT else 1
            if zstate["mm_u"] < zstate["t_u"] - lag:
                emit_z_mms(zstate["mm_u"])
                zstate["mm_u"] += 1

        def zsteps(n):
            for _ in range(n):
                if zstate["mm_u"] >= NUNIT:
                    return
                zstep()

        def z_barrier(n):
            # all z matmuls (and z_s evacuations) through unit n-1 emitted
            while zstate["mm_u"] < n:
                zstep()

        # ---- kick off: first bias chunk, then the small s DMA; z work
        # starts as soon as the first 16 j land ----
        emit_chunk_dma(0)
        snat = small_p.tile([P, CS], bf16, tag="snat")
        nc.gpsimd.dma_start(snat, s_c)
        zsteps(2)

        # ---- sT: transpose s_c to [c, i] ----
        sT = small_p.tile([P, KC, NI], bf16, tag="sT")
        for ch in range(2):
            tb = tpsum.tile([P, 1024], bf16, tag="tb", name=f"stb_{ch}")
            for co in range(ch * 4, ch * 4 + 4):
                nc.tensor.transpose(
                    tb[:, (co % 4) * P : (co % 4 + 1) * P],
                    snat[:, co * P : (co + 1) * P],
                    ident,
                )
            nc.vector.tensor_copy(sT[:, ch * 4 : (ch + 1) * 4, :], tb[:, : 4 * P])

        zsteps(2)

        # ---- weights: JIT 2 MB half-loads, PE-transpose [f,c]->[c,f] ----
        def emit_weight(w_ap, wT, name, zk=1):
            wr = w_ap.rearrange("(fo p) c -> p fo c", p=P)
            for half in range(2):
                wnat = wnat_p.tile([P, 4, CS], bf16, tag="wnat", name=f"{name}_{half}")
                nc.gpsimd.dma_start(wnat, wr[:, half * 4 : (half + 1) * 4, :])
                for fl in range(4):
                    fo = half * 4 + fl
                    tb = tpsum.tile([P, 1024], bf16, tag="tb", name=f"wtb_{name}_{fo}")
                    for co in range(KC):
                        nc.tensor.transpose(
                            tb[:, co * P : (co + 1) * P],
                            wnat[:, fl, co * P : (co + 1) * P],
                            ident,
                        )
                    # tb holds [c_sub, co(8) x 128f]; scatter to wT[:, co, fo*128..]
                    nc.vector.tensor_copy(
                        wT[:, :, fo * P : (fo + 1) * P],
                        tb.rearrange("p (a b) -> p a b", a=KC),
                    )
                    if fo % 2 == 1:
                        zsteps(zk)

        # ---- Wq + Q projection: qT[f,i] = (Wq s^T + bq)/sqrt(D) ----
        wqT = wt_p.tile([P, KC, CS], bf16, tag="wt", name="wqT")
        qT = small_p.tile([P, KC, NI], bf16, tag="qT")
        emit_weight(w_q, wqT, "wq")
        for fo in range(KC):
            ps = bigps.tile([P, 512], f32, tag="big", name=f"qp_{fo}")
            for co in range(KC):
                nc.tensor.matmul(
                    ps[:, :NI],
                    wqT[:, co, fo * P : (fo + 1) * P],
                    sT[:, co, :],
                    start=(co == 0),
                    stop=(co == KC - 1),
                )
            nc.vector.tensor_scalar(
                qT[:, fo, :],
                ps[:, :NI],
                bq_s[:, fo : fo + 1],
                1.0 / np.sqrt(D),
                ALU.add,
                ALU.mult,
            )
            if fo % 2 == 1:
                zsteps(1)

        # ---- kinT: transpose k_in to [c, j] ----
        knat = kin_p.tile([P, KC, CS], bf16, tag="knat")
        kr = k_in.rearrange("(jo p) c -> p jo c", p=P)
        nc.gpsimd.dma_start(knat[:, :4, :], kr[:, :4, :])
        nc.gpsimd.dma_start(knat[:, 4:, :], kr[:, 4:, :])
        kinT = kin_p.tile([P, KC, J], bf16, tag="kinT")
        for co in range(KC):
            tb = tpsum.tile([P, 1024], bf16, tag="tb", name=f"ktb_{co}")
            for jo in range(KC):
                nc.tensor.transpose(
                    tb[:, jo * P : (jo + 1) * P],
                    knat[:, jo, co * P : (co + 1) * P],
                    ident,
                )
            copy_alt(kinT[:, co, :], tb)
            if co % 2 == 1:
                zsteps(1)

        # ---- Wk + K projection: kT[f,j] = Wk k_in^T ----
        wkT = wt_p.tile([P, KC, CS], bf16, tag="wt", name="wkT")
        kT = big_p.tile([P, KC, J], bf16, tag="kT")
        emit_weight(w_k, wkT, "wk")

        def emit_k_proj(fo, jh):
            ps = bigps.tile([P, 512], f32, tag="big", name=f"kp_{fo}_{jh}")
            for co in range(KC):
                nc.tensor.matmul(
                    ps,
                    wkT[:, co, fo * P : (fo + 1) * P],
                    kinT[:, co, jh * 512 : (jh + 1) * 512],
                    start=(co == 0),
                    stop=(co == KC - 1),
                )
            copy_alt(kT[:, fo, jh * 512 : (jh + 1) * 512], ps)

        for fo in range(KC):  # j half 0 first: unblocks attn pass 0
            emit_k_proj(fo, 0)
            zsteps(1)

        # ---- Wv + V projection: v[j, h, d|mask] ----
        wvT = wt_p.tile([P, KC, CS], bf16, tag="wt", name="wvT")
        v_s = big_p.tile([P, KC, H, D + 1], bf16, tag="v")
        emit_weight(w_v, wvT, "wv")

        def emit_v_proj(jo):
            for fh in range(2):
                ps = bigps.tile([P, 512], f32, tag="big", name=f"vp_{jo}_{fh}")
                for co in range(KC):
                    nc.tensor.matmul(
                        ps,
                        kinT[:, co, jo * P : (jo + 1) * P],
                        wvT[:, co, fh * 512 : (fh + 1) * 512],
                        start=(co == 0),
                        stop=(co == KC - 1),
                    )
                nc.vector.tensor_scalar_mul(
                    v_s[:, jo, fh * 8 : (fh + 1) * 8, 0:D],
                    ps,
                    mask_s[:, jo : jo + 1],
                )
            nc.vector.tensor_copy(
                v_s[:, jo, :, D : D + 1],
                mask_s[:, jo : jo + 1, None].to_broadcast((P, H, 1)),
            )

        for jo in range(4):
            emit_v_proj(jo)
            zsteps(1)

        for fo in range(KC):
            emit_k_proj(fo, 1)
            zsteps(1)

        for jo in range(4, 8):
            emit_v_proj(jo)
            zsteps(1)

        # ---- Wg + G projection: g = sigmoid(s Wg^T) ----
        wgT = wt_p.tile([P, KC, CS], bf16, tag="wt", name="wgT")
        g_s = small_p.tile([P, CS], bf16, tag="g")
        emit_weight(w_g, wgT, "wg")
        for fh in range(2):
            ps = bigps.tile([P, 512], f32, tag="big", name=f"gp_{fh}")
            for co in range(KC):
                nc.tensor.matmul(
                    ps,
                    sT[:, co, :],
                    wgT[:, co, fh * 512 : (fh + 1) * 512],
                    start=(co == 0),
                    stop=(co == KC - 1),
                )
            nc.scalar.activation(g_s[:, fh * 512 : (fh + 1) * 512], ps, AF.Sigmoid)
            zsteps(1)

        # ---- Wo transpose (consumed only at the tail) ----
        woT = wt_p.tile([P, KC, CS], bf16, tag="wt", name="woT")
        emit_weight(w_o, woT, "wo", zk=2)

        # ---- attention: 3 passes over j (512, 256, 256) ----
        o_s = small_p.tile([P, CS], bf16, tag="o")
        o_acc = small_p.tile([P, H, D + 1], f32, tag="oacc")

        def emit_attn_h(jq, j0, nj, h, zk):
            fo, pb = h // 2, (h % 2) * D
            qkt = qkps.tile([P, 512], f32, tag="qk", name=f"qk_{jq}_{h}")
            qk = qkt[:, :nj]
            nc.tensor.matmul(
                qk,
                qT[pb : pb + D, fo, :],
                kT[pb : pb + D, fo, j0 : j0 + nj],
                start=True,
                stop=True,
            )
            # add z in place in PSUM, then exp on ACT reading PSUM; the z
            # units emitted in between fill the PE while add+exp run
            nc.vector.tensor_tensor(
                qk,
                qk,
                z_s[:, j0 // CJ : (j0 + nj) // CJ, :, h].rearrange("p a b -> p (a b)"),
                ALU.add,
            )
            et = et_p.tile([P, 512], bf16, tag="et", name=f"et_{jq}_{h}")
            nc.scalar.activation(et[:, :nj], qk, AF.Exp)
            zsteps(zk)
            tb = tpsum.tile([P, 1024], bf16, tag="tb", name=f"etb_{jq}_{h}")
            nt = nj // P
            for jl in range(nt):
                nc.tensor.transpose(
                    tb[:, jl * P : (jl + 1) * P], et[:, jl * P : (jl + 1) * P], ident
                )
            etT = et_p.tile([P, 4, P], bf16, tag="etT", name=f"etT_{jq}_{h}")
            nc.vector.tensor_copy(etT[:, :nt, :], tb[:, : nt * P])
            zsteps(zk)
            op = ops.tile([P, 512], f32, tag="op", name=f"op_{jq}_{h}")
            for q in range(nt):
                nc.tensor.matmul(
                    op[:, : D + 1],
                    etT[:, q, :],
                    v_s[:, j0 // P + q, h, :],
                    start=(q == 0),
                    stop=(q == nt - 1),
                )
            if jq == 0:
                nc.vector.tensor_copy(o_acc[:, h, :], op[:, : D + 1])
            else:
                nc.vector.tensor_tensor(
                    o_acc[:, h, :], op[:, : D + 1], o_acc[:, h, :], ALU.add
                )

        # pass 0: j 0..511 (needs z chunks 0-15, kT jh0, v jo0-3)
        z_barrier(64)
        for h in range(H):
            emit_attn_h(0, 0, 512, h, 1)
        # pass 1: j 512..767 (needs z chunks 16-23, kT jh1, v jo4-5)
        z_barrier(96)
        for h in range(H):
            emit_attn_h(1, 512, 256, h, 1)
        # pass 2: j 768..1023
        z_barrier(NUNIT)
        for h in range(H):
            emit_attn_h(2, 768, 256, h, 0)

        # ---- normalize: o = o_num / o_den ----
        for h in range(H):
            rec = r_p.tile([P, 1], f32, tag="r", name=f"rec_{h}")
            nc.vector.reciprocal(rec, o_acc[:, h, D : D + 1])
            nc.vector.tensor_scalar_mul(
                o_s[:, h * D : (h + 1) * D], o_acc[:, h, 0:D], rec
            )

        # ---- gating + output projection ----
        nc.vector.tensor_mul(g_s, g_s, o_s)
        goT = small_p.tile([P, KC, NI], bf16, tag="goT")
        for gh in range(2):
            tb = tpsum.tile([P, 1024], bf16, tag="tb", name=f"gtb_{gh}")
            for fo in range(gh * 4, gh * 4 + 4):
                nc.tensor.transpose(
                    tb[:, (fo % 4) * P : (fo % 4 + 1) * P],
                    g_s[:, fo * P : (fo + 1) * P],
                    ident,
                )
            nc.vector.tensor_copy(goT[:, gh * 4 : (gh + 1) * 4, :], tb[:, : 4 * P])

        for fh in range(2):
            ps = bigps.tile([P, 512], f32, tag="big", name=f"op_ps_{fh}")
            for fo in range(KC):
                nc.tensor.matmul(
                    ps,
                    goT[:, fo, :],
                    woT[:, fo, fh * 512 : (fh + 1) * 512],
                    start=(fo == 0),
                    stop=(fo == KC - 1),
                )
            out_s = outs_p.tile([P, 512], f32, tag="outs", name=f"out_s{fh}")
            nc.vector.tensor_copy(out_s, ps)
            nc.sync.dma_start(out_d[:, fh * 512 : (fh + 1) * 512], out_s)

    nc.compile()
    return nc


def kernel(**inputs):
    global _last_results
    from concourse.bass_utils import run_bass_kernel_spmd

    s = np.ascontiguousarray(np.asarray(inputs["s"], dtype=np.float32)[0])
    k_in = np.ascontiguousarray(np.asarray(inputs["k_in"], dtype=np.float32)[0])
    mask = np.ascontiguousarray(np.asarray(inputs["mask"], dtype=np.float32)[0])
    bias = np.asarray(inputs["bias"], dtype=np.float32)[0]
    wq = np.ascontiguousarray(np.asarray(inputs["Wq"], dtype=np.float32))
    wk = np.ascontiguousarray(np.asarray(inputs["Wk"], dtype=np.float32))
    wv = np.ascontiguousarray(np.asarray(inputs["Wv"], dtype=np.float32))
    wg = np.ascontiguousarray(np.asarray(inputs["Wg"], dtype=np.float32))
    wo = np.ascontiguousarray(np.asarray(inputs["Wo"], dtype=np.float32))
    bq = np.ascontiguousarray(np.asarray(inputs["bq"], dtype=np.float32))
    wz = np.ascontiguousarray(np.asarray(inputs["Wz"], dtype=np.float32))
    mult = int(np.asarray(inputs.get("multiplicity", 1)))
    assert mult == 1, f"multiplicity={mult} not supported (B=1)"

    nc = _build_program()

    in_maps = []
    for c in range(NCORES):
        in_maps.append(
            {
                "s_c": np.ascontiguousarray(s[c * NI : (c + 1) * NI]),
                "bias_c": np.ascontiguousarray(bias[c * NI : (c + 1) * NI]),
                "k_in": k_in,
                "mask": mask,
                "w_q": wq,
                "w_k": wk,
                "w_v": wv,
                "w_g": wg,
                "w_o": wo,
                "b_q": bq,
                "w_z": wz,
            }
        )

    try:
        res = run_bass_kernel_spmd(nc, in_maps, core_ids=list(range(NCORES)))
    except Exception:
        # transient device-unrecoverable errors have been observed on a
        # first attempt; one retry has always succeeded
        import time as _time

        _time.sleep(5.0)
        res = run_bass_kernel_spmd(nc, in_maps, core_ids=list(range(NCORES)))
    _last_results = res
    out = np.concatenate([r["out"] for r in res.results], axis=0)
    return out.reshape(B, I, CS).astype(np.float32)


if __name__ == "__main__":
    rng = np.random.default_rng(0)
    ins = {
        "s": rng.standard_normal((B, I, CS), dtype=np.float32),
        "k_in": rng.standard_normal((B, J, CS), dtype=np.float32),
        "mask": np.ones((B, J), np.float32),
        "bias": rng.standard_normal((B, I, J, CZ), dtype=np.float32),
        "Wq": rng.standard_normal((CS, CS), dtype=np.float32) * 0.02,
        "bq": rng.standard_normal((CS,), dtype=np.float32) * 0.02,
        "Wk": rng.standard_normal((CS, CS), dtype=np.float32) * 0.02,
        "Wv": rng.standard_normal((CS, CS), dtype=np.float32) * 0.02,
        "Wg": rng.standard_normal((CS, CS), dtype=np.float32) * 0.02,
        "Wo": rng.standard_normal((CS, CS), dtype=np.float32) * 0.02,
        "Wz": rng.standard_normal((CZ, H), dtype=np.float32) * 0.02,
        "multiplicity": 1,
    }
    out = kernel(**ins)
    print(out.shape, out.dtype)


# revision 17
# speedup vs baseline: 1.3469x; 1.1002x over previous
# Trainium2 Bass kernel for nn_CrossAttention (B=1, I=J=1024, C_S=1024,
# C_Z=128, H=16, D=64), sharded over the query dim i across 8 NeuronCores.
#
# Per-core program (i-slice of 128 query rows):
#   qT = (Wq s_c^T + bq)/sqrt(D)  kT = Wk k_in^T   v = k_in Wv^T  (bf16 matmuls)
#   z[i,j,h] = sum_c bias[i,j,c] Wz[c,h]   via PE-transpose of bias [i,c] blocks
#              (bias cast to bf16 during DMA) + per-j matmuls with Wz
#   qk[i,j] = qT_h^T kT_h  (PSUM), z added in-place in PSUM, exp on ACT
#              (softmax over j without max-subtraction; logits are O(1))
#   o[i,:] = sum_j exp^T v_aug[j]  with mask[j] in an extra v column so the
#              denominator comes out of the same matmul chain
#   out = (sigmoid(s_c Wg^T) * o) @ Wo^T
#
# The z-path (1024 PE transposes + 1024 small matmuls) is interleaved at fine
# granularity with the projection matmuls and 3 attention passes (512/256/256
# j), while the 64 MB/core bias DMA streams continuously underneath.  Weight
# DMAs are issued just-in-time in 2 MB halves so they do not starve the bias
# stream early on.
#
# kernel(**inputs) takes FULL inputs, shards on host, runs SPMD on cores 0-7,
# gathers to the full [1, 1024, 1024] output.

import numpy as np

B, I, J, CS, CZ, H, D = 1, 1024, 1024, 1024, 128, 16, 64
NCORES = 8
NI = I // NCORES  # 128 query rows per core
P = 128
NCHUNK = 32  # bias chunks of 32 j
CJ = J // NCHUNK  # 32 j per chunk
NUNIT = J // 8  # z work units of 8 j

_last_results = None


def _build_program():
    from contextlib import ExitStack

    import concourse.mybir as mybir
    import concourse.tile as tile
    from concourse import bacc
    from concourse.masks import make_identity

    f32 = mybir.dt.float32
    bf16 = mybir.dt.bfloat16
    AF = mybir.ActivationFunctionType
    ALU = mybir.AluOpType

    nc = bacc.Bacc("TRN2", target_bir_lowering=False, debug=False)

    # ---- dram io ----
    # bulk inputs are pre-cast to bf16 on the host (they are consumed as
    # bf16 on-chip anyway), halving HBM read traffic
    s_c = nc.dram_tensor("s_c", [NI, CS], bf16, kind="ExternalInput").ap()
    bias_c = nc.dram_tensor("bias_c", [NI, J, CZ], bf16, kind="ExternalInput").ap()
    k_in = nc.dram_tensor("k_in", [J, CS], bf16, kind="ExternalInput").ap()
    mask = nc.dram_tensor("mask", [J], f32, kind="ExternalInput").ap()
    w_q = nc.dram_tensor("w_q", [CS, CS], bf16, kind="ExternalInput").ap()
    w_k = nc.dram_tensor("w_k", [CS, CS], bf16, kind="ExternalInput").ap()
    w_v = nc.dram_tensor("w_v", [CS, CS], bf16, kind="ExternalInput").ap()
    w_g = nc.dram_tensor("w_g", [CS, CS], bf16, kind="ExternalInput").ap()
    w_o = nc.dram_tensor("w_o", [CS, CS], bf16, kind="ExternalInput").ap()
    b_q = nc.dram_tensor("b_q", [CS], f32, kind="ExternalInput").ap()
    w_z = nc.dram_tensor("w_z", [CZ, H], bf16, kind="ExternalInput").ap()
    out_d = nc.dram_tensor("out", [NI, CS], f32, kind="ExternalOutput").ap()

    KC = CS // P  # 8 contraction chunks

    with tile.TileContext(nc) as tc, ExitStack() as ctx:
        pool = lambda name, bufs: ctx.enter_context(tc.tile_pool(name=name, bufs=bufs))
        ppool = lambda name, bufs: ctx.enter_context(
            tc.tile_pool(name=name, bufs=bufs, space="PSUM")
        )

        const = pool("const", 1)
        wnat_p = pool("wnat", 2)
        wt_p = pool("wt", 2)
        kin_p = pool("kin", 1)
        small_p = pool("small", 1)
        big_p = pool("big", 1)
        bstage_p = pool("bstage", 3)
        bt_p = pool("bt", 3)
        et_p = pool("et", 2)
        r_p = pool("r", 2)
        outs_p = pool("outs", 1)

        tpsum = ppool("tpsum", 2)  # transpose targets (1 bank each)
        zps = ppool("zps", 1)  # z accumulation [128,512] f32
        bigps = ppool("bigps", 2)  # projection accumulators
        qkps = ppool("qkps", 2)  # attention logits f32
        ops = ppool("ops", 1)  # attention output f32

        # alternate big PSUM evacuations 2:1 between DVE and ACT (ACT SBUF
        # writes are ~1.9x slower than DVE's)
        engflip = [0]

        def copy_alt(out, in_):
            engflip[0] = (engflip[0] + 1) % 3
            if engflip[0] == 0:
                nc.scalar.copy(out, in_)
            else:
                nc.vector.tensor_copy(out, in_)

        ident = const.tile([P, P], bf16)
        make_identity(nc, ident)
        wz_s = const.tile([CZ, H], bf16)
        nc.sync.dma_start(wz_s, w_z)
        bq_s = const.tile([P, KC], f32)
        nc.sync.dma_start(bq_s, b_q.rearrange("(fo p) -> p fo", p=P))
        mask_s = const.tile([P, KC], f32)
        nc.sync.dma_start(mask_s, mask.rearrange("(jo p) -> p jo", p=P))

        # ---- z state ----
        # z_s layout: [i_part, chunk(32), j_local(32), h(16)] bf16
        z_s = big_p.tile([P, NCHUNK, CJ, H], bf16, tag="z")
        bstage = [None] * NCHUNK

        def emit_chunk_dma(c):
            bt = bstage_p.tile([P, CJ, CZ], bf16, tag="bt", name=f"bt_{c}")
            bstage[c] = bt
            if c == 0:
                # split the first chunk so the PE can start early
                nc.sync.dma_start(bt[:, :16, :], bias_c[:, :16, :])
                nc.sync.dma_start(bt[:, 16:, :], bias_c[:, 16:CJ, :])
            else:
                nc.sync.dma_start(bt, bias_c[:, c * CJ : (c + 1) * CJ, :])

        # software-pipelined z stream: transposes of unit u run while the
        # z-matmuls of unit u-1 (whose bT copy is by then complete) issue,
        # so the in-order PE queue never waits on a PSUM->SBUF copy.
        zstate = {"t_u": 0, "mm_u": 0, "zp": None, "bT": {}}

        def emit_z_transposes(u):
            c, uu = divmod(u, 4)  # chunk, unit-in-chunk
            if uu == 0:
                if bstage[c] is None:
                    emit_chunk_dma(c)
                if c + 1 < NCHUNK and bstage[c + 1] is None:
                    emit_chunk_dma(c + 1)
            bt = bstage[c]
            tb = tpsum.tile([P, 1024], bf16, tag="tb", name=f"ztb_{u}")
            for jl in range(8):
                nc.tensor.transpose(
                    tb[:, jl * P : (jl + 1) * P], bt[:, uu * 8 + jl, :], ident
                )
            bT = bt_p.tile([P, 8, P], bf16, tag="bT", name=f"bT_{u}")
            copy_alt(bT, tb)
            zstate["bT"][u] = bT

        def emit_z_mms(u):
            c, uu = divmod(u, 4)
            if uu == 0:
                zstate["zp"] = zps.tile([P, 512], f32, tag="zp", name=f"zp_{u}")
            zp = zstate["zp"]
            bT = zstate["bT"].pop(u)
            for jl in range(8):
                jj = uu * 8 + jl
                nc.tensor.matmul(
                    zp[:, jj * H : (jj + 1) * H],
                    bT[:, jl, :],
                    wz_s,
                    start=True,
                    stop=True,
                )
            if uu == 3:
                nc.scalar.copy(z_s[:, c, :, :].rearrange("p a b -> p (a b)"), zp)

        def zstep():
            if zstate["t_u"] < NUNIT:
                emit_z_transposes(zstate["t_u"])
                zstate["t_u"] += 1
            lag = 0 if zstate["t_u"] == NUNIT else 1
            if zstate["mm_u"] < zstate["t_u"] - lag:
                emit_z_mms(zstate["mm_u"])
                zstate["mm_u"] += 1

        def zsteps(n):
            for _ in range(n):
                if zstate["mm_u"] >= NUNIT:
                    return
                zstep()

        def z_barrier(n):
            # all z matmuls (and z_s evacuations) through unit n-1 emitted
            while zstate["mm_u"] < n:
                zstep()

        # ---- kick off: first bias chunk, then the small s DMA; z work
        # starts as soon as the first 16 j land ----
        emit_chunk_dma(0)
        snat = small_p.tile([P, CS], bf16, tag="snat")
        nc.sync.dma_start(snat, s_c)
        zsteps(2)

        # ---- sT: transpose s_c to [c, i] ----
        sT = small_p.tile([P, KC, NI], bf16, tag="sT")
        for ch in range(2):
            tb = tpsum.tile([P, 1024], bf16, tag="tb", name=f"stb_{ch}")
            for co in range(ch * 4, ch * 4 + 4):
                nc.tensor.transpose(
                    tb[:, (co % 4) * P : (co % 4 + 1) * P],
                    snat[:, co * P : (co + 1) * P],
                    ident,
                )
            nc.vector.tensor_copy(sT[:, ch * 4 : (ch + 1) * 4, :], tb[:, : 4 * P])

        zsteps(2)

        # ---- weights: JIT 2 MB half-loads, PE-transpose [f,c]->[c,f] ----
        def emit_weight(w_ap, wT, name, zk=1):
            wr = w_ap.rearrange("(fo p) c -> p fo c", p=P)
            for half in range(2):
                wnat = wnat_p.tile([P, 4, CS], bf16, tag="wnat", name=f"{name}_{half}")
                nc.sync.dma_start(wnat, wr[:, half * 4 : (half + 1) * 4, :])
                for fl in range(4):
                    fo = half * 4 + fl
                    tb = tpsum.tile([P, 1024], bf16, tag="tb", name=f"wtb_{name}_{fo}")
                    for co in range(KC):
                        nc.tensor.transpose(
                            tb[:, co * P : (co + 1) * P],
                            wnat[:, fl, co * P : (co + 1) * P],
                            ident,
                        )
                    # tb holds [c_sub, co(8) x 128f]; scatter to wT[:, co, fo*128..]
                    nc.vector.tensor_copy(
                        wT[:, :, fo * P : (fo + 1) * P],
                        tb.rearrange("p (a b) -> p a b", a=KC),
                    )
                    if fo % 2 == 1:
                        zsteps(zk)

        # ---- Wq + Q projection: qT[f,i] = (Wq s^T + bq)/sqrt(D) ----
        wqT = wt_p.tile([P, KC, CS], bf16, tag="wt", name="wqT")
        qT = small_p.tile([P, KC, NI], bf16, tag="qT")
        emit_weight(w_q, wqT, "wq")
        for fo in range(KC):
            ps = bigps.tile([P, 512], f32, tag="big", name=f"qp_{fo}")
            for co in range(KC):
                nc.tensor.matmul(
                    ps[:, :NI],
                    wqT[:, co, fo * P : (fo + 1) * P],
                    sT[:, co, :],
                    start=(co == 0),
                    stop=(co == KC - 1),
                )
            nc.vector.tensor_scalar(
                qT[:, fo, :],
                ps[:, :NI],
                bq_s[:, fo : fo + 1],
                1.0 / np.sqrt(D),
                ALU.add,
                ALU.mult,
            )
            if fo % 2 == 1:
                zsteps(1)

        # ---- kinT: transpose k_in to [c, j] ----
        knat = kin_p.tile([P, KC, CS], bf16, tag="knat")
        kr = k_in.rearrange("(jo p) c -> p jo c", p=P)
        nc.sync.dma_start(knat[:, :4, :], kr[:, :4, :])
        nc.sync.dma_start(knat[:, 4:, :], kr[:, 4:, :])
        kinT = kin_p.tile([P, KC, J], bf16, tag="kinT")
        for co in range(KC):
            tb = tpsum.tile([P, 1024], bf16, tag="tb", name=f"ktb_{co}")
            for jo in range(KC):
                nc.tensor.transpose(
                    tb[:, jo * P : (jo + 1) * P],
                    knat[:, jo, co * P : (co + 1) * P],
                    ident,
                )
            copy_alt(kinT[:, co, :], tb)
            if co % 2 == 1:
                zsteps(1)

        # ---- Wk + K projection: kT[f,j] = Wk k_in^T ----
        wkT = wt_p.tile([P, KC, CS], bf16, tag="wt", name="wkT")
        kT = big_p.tile([P, KC, J], bf16, tag="kT")
        emit_weight(w_k, wkT, "wk")

        def emit_k_proj(fo, jh):
            ps = bigps.tile([P, 512], f32, tag="big", name=f"kp_{fo}_{jh}")
            for co in range(KC):
                nc.tensor.matmul(
                    ps,
                    wkT[:, co, fo * P : (fo + 1) * P],
                    kinT[:, co, jh * 512 : (jh + 1) * 512],
                    start=(co == 0),
                    stop=(co == KC - 1),
                )
            copy_alt(kT[:, fo, jh * 512 : (jh + 1) * 512], ps)

        for fo in range(KC):  # j half 0 first: unblocks attn pass 0
            emit_k_proj(fo, 0)
            zsteps(1)

        # ---- Wv + V projection: v[j, h, d|mask] ----
        wvT = wt_p.tile([P, KC, CS], bf16, tag="wt", name="wvT")
        v_s = big_p.tile([P, KC, H, D + 1], bf16, tag="v")
        emit_weight(w_v, wvT, "wv")

        def emit_v_proj(jo):
            for fh in range(2):
                ps = bigps.tile([P, 512], f32, tag="big", name=f"vp_{jo}_{fh}")
                for co in range(KC):
                    nc.tensor.matmul(
                        ps,
                        kinT[:, co, jo * P : (jo + 1) * P],
                        wvT[:, co, fh * 512 : (fh + 1) * 512],
                        start=(co == 0),
                        stop=(co == KC - 1),
                    )
                nc.vector.tensor_scalar_mul(
                    v_s[:, jo, fh * 8 : (fh + 1) * 8, 0:D],
                    ps,
                    mask_s[:, jo : jo + 1],
                )
            nc.vector.tensor_copy(
                v_s[:, jo, :, D : D + 1],
                mask_s[:, jo : jo + 1, None].to_broadcast((P, H, 1)),
            )

        for jo in range(4):
            emit_v_proj(jo)
            zsteps(1)

        for fo in range(KC):
            emit_k_proj(fo, 1)
            zsteps(1)

        for jo in range(4, 8):
            emit_v_proj(jo)
            zsteps(1)

        # ---- Wg + G projection: g = sigmoid(s Wg^T) ----
        wgT = wt_p.tile([P, KC, CS], bf16, tag="wt", name="wgT")
        g_s = small_p.tile([P, CS], bf16, tag="g")
        emit_weight(w_g, wgT, "wg")
        for fh in range(2):
            ps = bigps.tile([P, 512], f32, tag="big", name=f"gp_{fh}")
            for co in range(KC):
                nc.tensor.matmul(
                    ps,
                    sT[:, co, :],
                    wgT[:, co, fh * 512 : (fh + 1) * 512],
                    start=(co == 0),
                    stop=(co == KC - 1),
                )
            nc.scalar.activation(g_s[:, fh * 512 : (fh + 1) * 512], ps, AF.Sigmoid)
            zsteps(1)

        # ---- Wo transpose (consumed only at the tail) ----
        woT = wt_p.tile([P, KC, CS], bf16, tag="wt", name="woT")
        emit_weight(w_o, woT, "wo", zk=2)

        # ---- attention: 3 passes over j (512, 256, 256) ----
        o_s = small_p.tile([P, CS], bf16, tag="o")
        o_acc = small_p.tile([P, H, D + 1], f32, tag="oacc")

        def emit_attn_h(jq, j0, nj, h, zk):
            fo, pb = h // 2, (h % 2) * D
            qkt = qkps.tile([P, 512], f32, tag="qk", name=f"qk_{jq}_{h}")
            qk = qkt[:, :nj]
            nc.tensor.matmul(
                qk,
                qT[pb : pb + D, fo, :],
                kT[pb : pb + D, fo, j0 : j0 + nj],
                start=True,
                stop=True,
            )
            # add z in place in PSUM, then exp on ACT reading PSUM; the z
            # units emitted in between fill the PE while add+exp run
            nc.vector.tensor_tensor(
                qk,
                qk,
                z_s[:, j0 // CJ : (j0 + nj) // CJ, :, h].rearrange("p a b -> p (a b)"),
                ALU.add,
            )
            et = et_p.tile([P, 512], bf16, tag="et", name=f"et_{jq}_{h}")
            nc.scalar.activation(et[:, :nj], qk, AF.Exp)
            zsteps(zk)
            tb = tpsum.tile([P, 1024], bf16, tag="tb", name=f"etb_{jq}_{h}")
            nt = nj // P
            for jl in range(nt):
                nc.tensor.transpose(
                    tb[:, jl * P : (jl + 1) * P], et[:, jl * P : (jl + 1) * P], ident
                )
            etT = et_p.tile([P, 4, P], bf16, tag="etT", name=f"etT_{jq}_{h}")
            nc.vector.tensor_copy(etT[:, :nt, :], tb[:, : nt * P])
            zsteps(zk)
            op = ops.tile([P, 512], f32, tag="op", name=f"op_{jq}_{h}")
            for q in range(nt):
                nc.tensor.matmul(
                    op[:, : D + 1],
                    etT[:, q, :],
                    v_s[:, j0 // P + q, h, :],
                    start=(q == 0),
                    stop=(q == nt - 1),
                )
            if jq == 0:
                nc.vector.tensor_copy(o_acc[:, h, :], op[:, : D + 1])
            else:
                nc.vector.tensor_tensor(
                    o_acc[:, h, :], op[:, : D + 1], o_acc[:, h, :], ALU.add
                )

        # pass 0: j 0..511 (needs z chunks 0-15, kT jh0, v jo0-3)
        z_barrier(64)
        for h in range(H):
            emit_attn_h(0, 0, 512, h, 1)
        # pass 1: j 512..767 (needs z chunks 16-23, kT jh1, v jo4-5)
        z_barrier(96)
        for h in range(H):
            emit_attn_h(1, 512, 256, h, 1)
        # pass 2: j 768..1023
        z_barrier(NUNIT)
        for h in range(H):
            emit_attn_h(2, 768, 256, h, 0)

        # ---- normalize: o = o_num / o_den ----
        for h in range(H):
            rec = r_p.tile([P, 1], f32, tag="r", name=f"rec_{h}")
            nc.vector.reciprocal(rec, o_acc[:, h, D : D + 1])
            nc.vector.tensor_scalar_mul(
                o_s[:, h * D : (h + 1) * D], o_acc[:, h, 0:D], rec
            )

        # ---- gating + output projection ----
        nc.vector.tensor_mul(g_s, g_s, o_s)
        goT = small_p.tile([P, KC, NI], bf16, tag="goT")
        for gh in range(2):
            tb = tpsum.tile([P, 1024], bf16, tag="tb", name=f"gtb_{gh}")
            for fo in range(gh * 4, gh * 4 + 4):
                nc.tensor.transpose(
                    tb[:, (fo % 4) * P : (fo % 4 + 1) * P],
                    g_s[:, fo * P : (fo + 1) * P],
                    ident,
                )
            nc.vector.tensor_copy(goT[:, gh * 4 : (gh + 1) * 4, :], tb[:, : 4 * P])

        for fh in range(2):
            ps = bigps.tile([P, 512], f32, tag="big", name=f"op_ps_{fh}")
            for fo in range(KC):
                nc.tensor.matmul(
                    ps,
                    goT[:, fo, :],
                    woT[:, fo, fh * 512 : (fh + 1) * 512],
                    start=(fo == 0),
                    stop=(fo == KC - 1),
                )
            out_s = outs_p.tile([P, 512], f32, tag="outs", name=f"out_s{fh}")
            nc.vector.tensor_copy(out_s, ps)
            nc.sync.dma_start(out_d[:, fh * 512 : (fh + 1) * 512], out_s)

    nc.compile()
    return nc


def kernel(**inputs):
    global _last_results
    import ml_dtypes

    from concourse.bass_utils import run_bass_kernel_spmd

    bf16 = ml_dtypes.bfloat16
    s = np.asarray(inputs["s"], dtype=np.float32)[0].astype(bf16)
    k_in = np.asarray(inputs["k_in"], dtype=np.float32)[0].astype(bf16)
    mask = np.ascontiguousarray(np.asarray(inputs["mask"], dtype=np.float32)[0])
    bias = np.asarray(inputs["bias"], dtype=np.float32)[0].astype(bf16)
    wq = np.asarray(inputs["Wq"], dtype=np.float32).astype(bf16)
    wk = np.asarray(inputs["Wk"], dtype=np.float32).astype(bf16)
    wv = np.asarray(inputs["Wv"], dtype=np.float32).astype(bf16)
    wg = np.asarray(inputs["Wg"], dtype=np.float32).astype(bf16)
    wo = np.asarray(inputs["Wo"], dtype=np.float32).astype(bf16)
    bq = np.ascontiguousarray(np.asarray(inputs["bq"], dtype=np.float32))
    wz = np.asarray(inputs["Wz"], dtype=np.float32).astype(bf16)
    mult = int(np.asarray(inputs.get("multiplicity", 1)))
    assert mult == 1, f"multiplicity={mult} not supported (B=1)"

    nc = _build_program()

    in_maps = []
    for c in range(NCORES):
        in_maps.append(
            {
                "s_c": np.ascontiguousarray(s[c * NI : (c + 1) * NI]),
                "bias_c": np.ascontiguousarray(bias[c * NI : (c + 1) * NI]),
                "k_in": k_in,
                "mask": mask,
                "w_q": wq,
                "w_k": wk,
                "w_v": wv,
                "w_g": wg,
                "w_o": wo,
                "b_q": bq,
                "w_z": wz,
            }
        )

    try:
        res = run_bass_kernel_spmd(nc, in_maps, core_ids=list(range(NCORES)))
    except Exception:
        # transient device-unrecoverable errors have been observed on a
        # first attempt; one retry has always succeeded
        import time as _time

        _time.sleep(5.0)
        res = run_bass_kernel_spmd(nc, in_maps, core_ids=list(range(NCORES)))
    _last_results = res
    out = np.concatenate([r["out"] for r in res.results], axis=0)
    return out.reshape(B, I, CS).astype(np.float32)


if __name__ == "__main__":
    rng = np.random.default_rng(0)
    ins = {
        "s": rng.standard_normal((B, I, CS), dtype=np.float32),
        "k_in": rng.standard_normal((B, J, CS), dtype=np.float32),
        "mask": np.ones((B, J), np.float32),
        "bias": rng.standard_normal((B, I, J, CZ), dtype=np.float32),
        "Wq": rng.standard_normal((CS, CS), dtype=np.float32) * 0.02,
        "bq": rng.standard_normal((CS,), dtype=np.float32) * 0.02,
        "Wk": rng.standard_normal((CS, CS), dtype=np.float32) * 0.02,
        "Wv": rng.standard_normal((CS, CS), dtype=np.float32) * 0.02,
        "Wg": rng.standard_normal((CS, CS), dtype=np.float32) * 0.02,
        "Wo": rng.standard_normal((CS, CS), dtype=np.float32) * 0.02,
        "Wz": rng.standard_normal((CZ, H), dtype=np.float32) * 0.02,
        "multiplicity": 1,
    }
    out = kernel(**ins)
    print(out.shape, out.dtype)
